# revision 10
# baseline (speedup 1.0000x reference)
"""GQA causal attention (RoPE) kernel for 8 TRN2 NeuronCores.

Sharding: core = b*4 + g  (b = batch 0..1, g = head-group 0..3).
Each core handles one batch element, 8 query heads (g*8..g*8+7) and the
2 KV heads (g*2, g*2+1) that serve them, plus the matching row-block of
Wo; per-core outputs are partial sums over the hidden dim that the host
reduces across the 4 groups of each batch.

On-core dataflow (all matmuls bf16 with f32 PSUM accumulation):
  QT = Wq_g.T @ X.T   [1024, 2048]   (feature-on-partition layout)
  KT = Wk_g.T @ X.T   [256, 2048]    + RoPE applied on DVE straight from
  VT = Wv_g.T @ X.T   PSUM (rotate_half = partition-shifted mul against a
                      sign-folded sin table); VT PE-free DMA-transposed to
                      V [2048, 256].
  per head, per q-128 chunk: S[q,k] = QT_chunk.T x KT (causal-narrowed),
  additive -1e9 mask on the diagonal 128-block via a DVE add into PSUM,
  P = exp(S/sqrt(d)) on ScalarE with fused row-sum (accum_out) -> per-row
  reciprocal -> P normalized in place; all 4 q-chunks of a head go out in
  ONE merged DMA transpose to P^T; ctx^T = V.T @ P^T; out_partial =
  ctx @ Wo_g (row block) accumulated over heads in PSUM.

Software pipeline: softmax(i) runs 2 heads ahead of AV(i); o-projection
groups of the previous q-block drip between AVs so the PE never drains
while a softmax chain (exp -> normalize -> P^T transpose) is in flight.
"""

import os

import numpy as np
import ml_dtypes

import concourse.bass as bass
import concourse.mybir as mybir
import concourse.tile as tile
from concourse import bacc
from concourse.bass_utils import run_bass_kernel_spmd
from contextlib import ExitStack

B, S, H = 2, 2048, 4096
NH, NKV, HD = 32, 8, 128
BASE = 10000.0
N_CORES = 8
GROUPS = 4
NH_L = NH // GROUPS        # 8 local q heads
NKV_L = NKV // GROUPS      # 2 local kv heads
HC = H // 128              # 32 hidden chunks
TC = S // 128              # 16 token chunks
TB = S // 512              # 4 token 512-blocks
OC = H // 512              # 8 output-feature 512-blocks
SCALE = 1.0 / float(np.sqrt(HD))
NEG = -1e9

BF16 = mybir.dt.bfloat16
F32 = mybir.dt.float32
EXP = mybir.ActivationFunctionType.Exp
AX = mybir.AxisListType.X
ADD = mybir.AluOpType.add

_PROG = None
LAST_EXEC_NS = None
LAST_RESULTS = None


def _build():
    nc = bacc.Bacc(None, target_bir_lowering=False, debug=False)
    with tile.TileContext(nc) as tc:
        xt_d = nc.dram_tensor("xt", [128, HC, S], BF16, kind="ExternalInput")
        wq_d = nc.dram_tensor("wq", [NH_L, 128, HC, 128], BF16, kind="ExternalInput")
        wk_d = nc.dram_tensor("wk", [NKV_L, 128, HC, 128], BF16, kind="ExternalInput")
        wv_d = nc.dram_tensor("wv", [NKV_L, 128, HC, 128], BF16, kind="ExternalInput")
        wo_d = nc.dram_tensor("wo", [NH_L, 128, H], BF16, kind="ExternalInput")
        cos_d = nc.dram_tensor("cos", [128, S], BF16, kind="ExternalInput")
        # sin2: rows 0-63 hold -sin[0:64], rows 64-127 hold sin[64:128], so
        # rotate_half(x)*sin == shiftedmul(x, sin2) with partition offsets
        sin2_d = nc.dram_tensor("sin2", [128, S], BF16, kind="ExternalInput")
        tria_d = nc.dram_tensor("tria", [128, 128], F32, kind="ExternalInput")
        out_d = nc.dram_tensor("out_p", [S, H], BF16, kind="ExternalOutput")

        with ExitStack() as stk:
            persist = stk.enter_context(tc.tile_pool(name="persist", bufs=1))
            q_all = persist.tile([128, NH_L, S], BF16, name="q_all", tag="q_all")
            k_all = persist.tile([128, NKV_L, S], BF16, name="k_all", tag="k_all")
            v_all = persist.tile([128, TC, NKV_L * 128], BF16, name="v_all", tag="v_all")
            tria_sb = persist.tile([128, 128], F32, name="tria_sb", tag="tria_sb")
            bias0 = persist.tile([128, 1], F32, name="bias0", tag="bias0")

            nc.sync.dma_start(out=tria_sb[:], in_=tria_d[:])
            nc.any.memset(bias0[:], 0.0)

            # one PSUM pool for the whole program:
            #   tag "acc" (4 banks): projection accumulators, attention AV,
            #                        o-proj accumulators
            #   tag "sp"  (4 banks): QK scores (two 512 k-blocks share one
            #                        tile so exp runs as one wide instruction)
            ps_pool = stk.enter_context(tc.tile_pool(name="ps", bufs=1, space="PSUM"))

            def acc_tile():
                return ps_pool.tile([128, 512], F32, name="acc", tag="acc", bufs=4)

            def sp_tile():
                return ps_pool.tile([128, 1024], F32, name="spt", tag="sp", bufs=2)

            # ---------------- projections ----------------
            with ExitStack() as proj:
                xt_pool = proj.enter_context(tc.tile_pool(name="xtp", bufs=1))

                def psum_quad(flip):
                    # alternate between the acc and sp PSUM tags so one
                    # projection accumulates while DVE rope / ScalarE copy
                    # still drains the previous one's PSUM
                    if flip:
                        s0, s1 = sp_tile(), sp_tile()
                        return [s0[:, :512], s0[:, 512:], s1[:, :512], s1[:, 512:]]
                    return [acc_tile()[:] for _ in range(TB)]

                # ---- V: both kv heads projected chunk-paired so the PE
                # consumes each fresh xt chunk twice and never outruns the
                # initial xt DMA stream ----
                with ExitStack() as vstk:
                    vw_pool = vstk.enter_context(tc.tile_pool(name="vwp", bufs=8))
                    raw_pool = vstk.enter_context(tc.tile_pool(name="rawp", bufs=2))
                    ws_v = []
                    for f in range(NKV_L):
                        for qtr in range(4):
                            wt = vw_pool.tile([128, 8, 128], BF16, name="vwt", tag="vwt")
                            nc.sync.dma_start(
                                out=wt[:], in_=wv_d[f, :, qtr * 8:(qtr + 1) * 8, :])
                            ws_v.append(wt)

                    xt_first = []
                    for i in range(2):
                        t = xt_pool.tile([128, 1, S], BF16, name=f"xtf{i}", tag=f"xtf{i}")
                        nc.sync.dma_start(out=t[:], in_=xt_d[:, i:i + 1, :])
                        xt_first.append(t)
                    xts = []
                    for i in range(1, 16):
                        t = xt_pool.tile([128, 2, S], BF16, name=f"xtt{i}", tag=f"xtt{i}")
                        nc.sync.dma_start(out=t[:], in_=xt_d[:, i * 2:(i + 1) * 2, :])
                        xts.append(t)

                    def xt_ap(hc, lo, hi):
                        if hc < 2:
                            return xt_first[hc][:, 0, lo:hi]
                        return xts[hc // 2 - 1][:, hc % 2, lo:hi]

                    pss_v = [psum_quad(False), psum_quad(True)]
                    for hc in range(HC):
                        for f in range(NKV_L):
                            lhsT = ws_v[f * 4 + hc // 8][:, hc % 8, :]
                            for tb in range(TB):
                                nc.tensor.matmul(
                                    pss_v[f][tb], lhsT,
                                    xt_ap(hc, tb * 512, (tb + 1) * 512),
                                    start=(hc == 0), stop=(hc == HC - 1),
                                )
                    for f in range(NKV_L):
                        raw = raw_pool.tile([128, S], BF16, name="raw", tag="raw")
                        for tb in range(TB):
                            nc.scalar.copy(raw[:, tb * 512:(tb + 1) * 512], pss_v[f][tb])
                        nc.sync.dma_start_transpose(
                            out=v_all[:, :, f * 128:(f + 1) * 128], in_=raw[:],
                        )

                # ---- K and Q, with RoPE applied straight from PSUM ----
                with ExitStack() as kq:
                    cs_pool = kq.enter_context(tc.tile_pool(name="csp", bufs=1))
                    cos_sb = cs_pool.tile([128, S], BF16, name="cos_sb", tag="cos_sb")
                    sin_sb = cs_pool.tile([128, S], BF16, name="sin_sb", tag="sin_sb")
                    nc.sync.dma_start(out=cos_sb[:], in_=cos_d[:])
                    nc.sync.dma_start(out=sin_sb[:], in_=sin2_d[:])
                    wpool = kq.enter_context(tc.tile_pool(name="wpool", bufs=4))
                    tmp_pool = kq.enter_context(tc.tile_pool(name="tmpp", bufs=2))

                    def project_T(w_d, f, flip):
                        ws = []
                        for qtr in range(4):
                            wt = wpool.tile([128, 8, 128], BF16, name="wt", tag="wt")
                            nc.sync.dma_start(
                                out=wt[:], in_=w_d[f, :, qtr * 8:(qtr + 1) * 8, :])
                            ws.append(wt)
                        pss = psum_quad(flip)
                        for hc in range(HC):
                            lhsT = ws[hc // 8][:, hc % 8, :]
                            for tb in range(TB):
                                nc.tensor.matmul(
                                    pss[tb], lhsT, xt_ap(hc, tb * 512, (tb + 1) * 512),
                                    start=(hc == 0), stop=(hc == HC - 1),
                                )
                        return pss

                    def rope_into(pss, dst, idx):
                        # dst = psum*cos + rot_half(psum)*sin; rot_half via
                        # partition-shifted reads of the PSUM tile against the
                        # sign-folded sin table (legal: one operand in PSUM)
                        for tb in range(TB):
                            sl = slice(tb * 512, (tb + 1) * 512)
                            ps = pss[tb]
                            t1 = tmp_pool.tile([128, 512], F32, name="t1", tag="t1")
                            t2 = tmp_pool.tile([128, 512], F32, name="t2", tag="t2")
                            nc.vector.tensor_mul(t1[:], ps, cos_sb[:, sl])
                            nc.vector.tensor_mul(t2[0:64, :], ps[64:128, :], sin_sb[0:64, sl])
                            nc.vector.tensor_mul(t2[64:128, :], ps[0:64, :], sin_sb[64:128, sl])
                            nc.vector.tensor_add(dst[:, idx, sl], t1[:], t2[:])

                    flip = False
                    for f in range(NKV_L):
                        rope_into(project_T(wk_d, f, flip), k_all, f)
                        flip = not flip
                    for f in range(NH_L):
                        rope_into(project_T(wq_d, f, flip), q_all, f)
                        flip = not flip

            # ---------------- attention + output projection ----------------
            with ExitStack() as att:
                wo_pool = att.enter_context(tc.tile_pool(name="wop", bufs=1))
                wo_sb = wo_pool.tile([128, NH_L, H], BF16, name="wo_sb", tag="wo_sb")
                for h in range(NH_L):
                    # SWDGE queues: keep the HWDGE queues free for the
                    # latency-critical P^T transposes
                    nc.gpsimd.dma_start(out=wo_sb[:, h, :], in_=wo_d[h])

                ct_pool = att.enter_context(tc.tile_pool(name="ctp", bufs=2))
                rs_pool = att.enter_context(tc.tile_pool(name="rsp", bufs=8))
                osb_pool = att.enter_context(tc.tile_pool(name="osbp", bufs=2))

                p_pool = att.enter_context(tc.tile_pool(name="pp", bufs=2))
                ptt_pool = att.enter_context(tc.tile_pool(name="pttp", bufs=2))

                cts_by_qb = {}

                def softmax_part(qb, h):
                    """QK + mask + exp + row-normalize + one merged P^T DMA."""
                    kv = h // (NH_L // NKV_L)
                    nkb = qb + 1
                    W = nkb * 512
                    p_sb = p_pool.tile([128, 4, W], BF16, name="p_sb", tag="p")
                    ptt = ptt_pool.tile([128, 4, nkb * 4, 128], BF16,
                                        name="ptt", tag="ptt")
                    for qcl in range(4):
                        qg = 4 * qb + qcl
                        Wq = (qg + 1) * 128
                        rs = rs_pool.tile([128, 2], F32, name="rs", tag="rs")
                        for kb2 in range(0, nkb, 2):
                            sp = sp_tile()
                            w0 = 0
                            for j in (0, 1):
                                kb = kb2 + j
                                if kb >= nkb:
                                    break
                                wk_ = 512 if kb < qb else Wq - kb * 512
                                nc.tensor.matmul(
                                    sp[:, j * 512:j * 512 + wk_],
                                    q_all[:, h, qg * 128:(qg + 1) * 128],
                                    k_all[:, kv, kb * 512:kb * 512 + wk_],
                                    start=True, stop=True,
                                )
                                if kb == qb:
                                    # additive causal mask for the diagonal
                                    # 128-block, applied on DVE into PSUM
                                    dsl = slice(j * 512 + wk_ - 128, j * 512 + wk_)
                                    nc.vector.tensor_add(sp[:, dsl], sp[:, dsl], tria_sb[:])
                                w0 += wk_
                            nc.scalar.activation(
                                p_sb[:, qcl, kb2 * 512:kb2 * 512 + w0], sp[:, :w0], EXP,
                                bias=bias0[:], scale=SCALE,
                                accum_out=rs[:, kb2 // 2:kb2 // 2 + 1],
                            )
                        nacc = (nkb + 1) // 2
                        rq = rs_pool.tile([128, 1], F32, name="rq", tag="rq")
                        if nacc > 1:
                            dsum = rs_pool.tile([128, 1], F32, name="dsum", tag="dsum")
                            nc.vector.tensor_reduce(dsum[:], rs[:, :nacc], axis=AX, op=ADD)
                            nc.vector.reciprocal(rq[:], dsum[:])
                        else:
                            nc.vector.reciprocal(rq[:], rs[:, :1])
                        nc.vector.tensor_scalar_mul(
                            p_sb[:, qcl, :Wq], p_sb[:, qcl, :Wq], rq[:])
                    # one merged transpose for all 4 q-chunks; chunks beyond a
                    # chunk's causal width hold stale garbage that the AV
                    # narrowing never reads
                    nc.sync.dma_start_transpose(out=ptt[:], in_=p_sb[:])
                    return ptt

                def av_part(qb, h, ptt):
                    kv = h // (NH_L // NKV_L)
                    nkc = 4 * (qb + 1)
                    av = acc_tile()
                    for kc in range(nkc):
                        d = max(0, kc - 4 * qb)
                        off = d * 128
                        nc.tensor.matmul(
                            av[:, off:512],
                            v_all[:, kc, kv * 128:(kv + 1) * 128],
                            ptt[:, d:4, kc, :],
                            start=(kc == 0), stop=(kc == nkc - 1),
                        )
                    nc.vector.tensor_copy(cts_by_qb[qb][:, h, :], av[:])

                osb_cur = [None]

                def oproj_group(qb, qcl, oc):
                    cts = cts_by_qb[qb]
                    qc = qb * 4 + qcl
                    op = acc_tile()
                    for h in range(NH_L):
                        nc.tensor.matmul(
                            op[:],
                            cts[:, h, qcl * 128:(qcl + 1) * 128],
                            wo_sb[:, h, oc * 512:(oc + 1) * 512],
                            start=(h == 0), stop=(h == NH_L - 1),
                        )
                    # batch 4 oc groups into one bf16 tile -> one out-DMA
                    if oc % 4 == 0:
                        osb_cur[0] = osb_pool.tile(
                            [128, 4, 512], BF16, name="osb", tag="osb")
                    osb = osb_cur[0]
                    nc.vector.tensor_copy(osb[:, oc % 4, :], op[:])
                    if oc % 4 == 3:
                        nc.gpsimd.dma_start(
                            out=out_d[qc * 128:(qc + 1) * 128,
                                      (oc - 3) * 512:(oc + 1) * 512],
                            in_=osb[:],
                        )

                # Software pipeline: softmax(i) runs 2 heads ahead of av(i);
                # o-projection groups of the previous (already AV-complete)
                # q-block drip between steps as PE filler so the exp ->
                # normalize -> P^T transpose chain never drains the PE.
                pairs = [(qb, h) for qb in (0, 1, 2, 3) for h in range(NH_L)]
                LOOK = 2
                DRIP = 4
                RESERVE = 4   # groups held back to pad the tail avs
                ptts = {}
                oproj_queue = []

                def emit_av(i):
                    qb, h = pairs[i]
                    av_part(qb, h, ptts.pop(i))
                    if h == NH_L - 1:
                        for qcl in range(4):
                            for oc in range(OC):
                                oproj_queue.append((qb, qcl, oc))

                def drip(n):
                    for _ in range(min(n, len(oproj_queue))):
                        pq, qcl, oc = oproj_queue.pop(0)
                        oproj_group(pq, qcl, oc)

                # per step: av(i-LOOK) + oproj filler FIRST on the PE queue,
                # then this step's QK — so the P^T buffer ring (which ties
                # tp(i) to av(i-LOOK)) unblocks early in the step instead of
                # stalling the transpose queue behind the whole step
                last_block = len(pairs) - NH_L
                for i, (qb, h) in enumerate(pairs):
                    if h == 0:
                        cts_by_qb[qb] = ct_pool.tile(
                            [128, NH_L, 512], BF16, name="cts", tag="ct")
                    if i >= LOOK:
                        emit_av(i - LOOK)
                        budget = DRIP
                        if i >= last_block:
                            budget = min(budget, len(oproj_queue) - RESERVE)
                        drip(max(0, budget))
                    ptts[i] = softmax_part(qb, h)
                for i in range(len(pairs) - LOOK, len(pairs)):
                    emit_av(i)
                    drip(RESERVE // LOOK)
                while oproj_queue:
                    pq, qcl, oc = oproj_queue.pop(0)
                    oproj_group(pq, qcl, oc)
    nc.compile()
    return nc


def _prep_inputs(hidden_states, position_ids, Wq, Wk, Wv, Wo):
    bf = ml_dtypes.bfloat16
    hidden_states = np.asarray(hidden_states, dtype=np.float32)
    position_ids = np.asarray(position_ids)
    Wq = np.asarray(Wq, dtype=np.float32)
    Wk = np.asarray(Wk, dtype=np.float32)
    Wv = np.asarray(Wv, dtype=np.float32)
    Wo = np.asarray(Wo, dtype=np.float32)

    inv_freq = (1.0 / (BASE ** (np.arange(0, HD, 2, dtype=np.float32) / HD))).astype(np.float32)
    ii = np.arange(128)
    tria = np.where(ii[None, :] > ii[:, None], np.float32(NEG), np.float32(0.0))

    per_batch = []
    for b in range(B):
        xt = np.ascontiguousarray(
            hidden_states[b].T.reshape(HC, 128, S).transpose(1, 0, 2)
        ).astype(bf)
        pos = position_ids[b].astype(np.float32)
        freqs = pos[:, None] * inv_freq[None, :]           # [S, 64]
        emb = np.concatenate([freqs, freqs], axis=1)       # [S, 128]
        cos = np.ascontiguousarray(np.cos(emb).T).astype(bf)
        sin = np.sin(emb).T                                # [128, S] f32
        sin2 = sin.copy()
        sin2[:64] = -sin[:64]                              # sign-folded
        sin2 = np.ascontiguousarray(sin2).astype(bf)
        per_batch.append((xt, cos, sin2))

    in_maps = []
    for core in range(N_CORES):
        b, g = core // GROUPS, core % GROUPS
        xt, cos, sin2 = per_batch[b]
        wq = np.ascontiguousarray(
            Wq[:, g * NH_L * HD:(g + 1) * NH_L * HD]
            .reshape(HC, 128, NH_L, 128).transpose(2, 1, 0, 3)
        ).astype(bf)
        wk = np.ascontiguousarray(
            Wk[:, g * NKV_L * HD:(g + 1) * NKV_L * HD]
            .reshape(HC, 128, NKV_L, 128).transpose(2, 1, 0, 3)
        ).astype(bf)
        wv = np.ascontiguousarray(
            Wv[:, g * NKV_L * HD:(g + 1) * NKV_L * HD]
            .reshape(HC, 128, NKV_L, 128).transpose(2, 1, 0, 3)
        ).astype(bf)
        wo = np.ascontiguousarray(
            Wo[g * NH_L * HD:(g + 1) * NH_L * HD, :].reshape(NH_L, 128, H)
        ).astype(bf)
        in_maps.append({
            "xt": xt, "wq": wq, "wk": wk, "wv": wv, "wo": wo,
            "cos": cos, "sin2": sin2, "tria": tria,
        })
    return in_maps


def kernel(hidden_states, position_ids, Wq, Wk, Wv, Wo):
    global _PROG, LAST_EXEC_NS, LAST_RESULTS
    if _PROG is None:
        _PROG = _build()
    nc = _PROG
    in_maps = _prep_inputs(hidden_states, position_ids, Wq, Wk, Wv, Wo)
    trace = os.environ.get("BASS_KERNEL_TRACE", "0") == "1"
    res = run_bass_kernel_spmd(nc, in_maps, core_ids=list(range(N_CORES)), trace=trace)
    LAST_EXEC_NS = res.exec_time_ns
    LAST_RESULTS = res
    out = np.zeros((B, S, H), dtype=np.float32)
    for core in range(N_CORES):
        out[core // GROUPS] += res.results[core]["out_p"].astype(np.float32)
    return out


# revision 15
# speedup vs baseline: 1.0913x; 1.0913x over previous
"""GQA causal attention (RoPE) kernel for 8 TRN2 NeuronCores.

Sharding: core = b*4 + g  (b = batch 0..1, g = head-group 0..3).
Each core handles one batch element, 8 query heads (g*8..g*8+7) and the
2 KV heads (g*2, g*2+1) that serve them, plus the matching row-block of
Wo; per-core outputs are partial sums over the hidden dim that the host
reduces across the 4 groups of each batch.

On-core dataflow (all matmuls bf16 with f32 PSUM accumulation):
  QT = Wq_g.T @ X.T   [1024, 2048]   (feature-on-partition layout)
  KT = Wk_g.T @ X.T   [256, 2048]    + RoPE applied on DVE straight from
  VT = Wv_g.T @ X.T   PSUM (rotate_half = partition-shifted mul against a
                      sign-folded sin table); VT PE-free DMA-transposed to
                      V [2048, 256].
  per head, per q-128 chunk: S[q,k] = QT_chunk.T x KT (causal-narrowed),
  additive -1e9 mask on the diagonal 128-block via a DVE add into PSUM,
  P = exp(S/sqrt(d)) on ScalarE with fused row-sum (accum_out) -> per-row
  reciprocal -> P normalized in place; all 4 q-chunks of a head go out in
  ONE merged DMA transpose to P^T; ctx^T = V.T @ P^T; out_partial =
  ctx @ Wo_g (row block) accumulated over heads in PSUM.

Software pipeline: softmax(i) runs 2 heads ahead of AV(i); o-projection
groups of the previous q-block drip between AVs so the PE never drains
while a softmax chain (exp -> normalize -> P^T transpose) is in flight.
"""

import os

import numpy as np
import ml_dtypes

import concourse.bass as bass
import concourse.mybir as mybir
import concourse.tile as tile
from concourse import bacc
from concourse.bass_utils import run_bass_kernel_spmd
from contextlib import ExitStack

B, S, H = 2, 2048, 4096
NH, NKV, HD = 32, 8, 128
BASE = 10000.0
N_CORES = 8
GROUPS = 4
NH_L = NH // GROUPS        # 8 local q heads
NKV_L = NKV // GROUPS      # 2 local kv heads
HC = H // 128              # 32 hidden chunks
TC = S // 128              # 16 token chunks
TB = S // 512              # 4 token 512-blocks
OC = H // 512              # 8 output-feature 512-blocks
SCALE = 1.0 / float(np.sqrt(HD))
NEG = -1e9

BF16 = mybir.dt.bfloat16
F32 = mybir.dt.float32
EXP = mybir.ActivationFunctionType.Exp
AX = mybir.AxisListType.X
ADD = mybir.AluOpType.add

_PROG = None
LAST_EXEC_NS = None
LAST_RESULTS = None


def _build():
    nc = bacc.Bacc(None, target_bir_lowering=False, debug=False)
    with tile.TileContext(nc) as tc:
        xt_d = nc.dram_tensor("xt", [128, HC, S], BF16, kind="ExternalInput")
        wq_d = nc.dram_tensor("wq", [NH_L, 128, HC, 128], BF16, kind="ExternalInput")
        wk_d = nc.dram_tensor("wk", [NKV_L, 128, HC, 128], BF16, kind="ExternalInput")
        wv_d = nc.dram_tensor("wv", [NKV_L, 128, HC, 128], BF16, kind="ExternalInput")
        wo_d = nc.dram_tensor("wo", [NH_L, 128, H], BF16, kind="ExternalInput")
        cos_d = nc.dram_tensor("cos", [128, S], BF16, kind="ExternalInput")
        # sin2: rows 0-63 hold -sin[0:64], rows 64-127 hold sin[64:128], so
        # rotate_half(x)*sin == shiftedmul(x, sin2) with partition offsets
        sin2_d = nc.dram_tensor("sin2", [128, S], BF16, kind="ExternalInput")
        tria_d = nc.dram_tensor("tria", [128, 128], F32, kind="ExternalInput")
        out_d = nc.dram_tensor("out_p", [S, H], BF16, kind="ExternalOutput")

        with ExitStack() as stk:
            persist = stk.enter_context(tc.tile_pool(name="persist", bufs=1))
            q_all = persist.tile([128, NH_L, S], BF16, name="q_all", tag="q_all")
            k_all = persist.tile([128, NKV_L, S], BF16, name="k_all", tag="k_all")
            v_all = persist.tile([128, TC, NKV_L * 128], BF16, name="v_all", tag="v_all")
            tria_sb = persist.tile([128, 128], F32, name="tria_sb", tag="tria_sb")
            bias0 = persist.tile([128, 1], F32, name="bias0", tag="bias0")

            nc.sync.dma_start(out=tria_sb[:], in_=tria_d[:])
            nc.any.memset(bias0[:], 0.0)

            # one PSUM pool for the whole program:
            #   tag "acc" (4 banks): projection accumulators, attention AV,
            #                        o-proj accumulators
            #   tag "sp"  (4 banks): QK scores (two 512 k-blocks share one
            #                        tile so exp runs as one wide instruction)
            ps_pool = stk.enter_context(tc.tile_pool(name="ps", bufs=1, space="PSUM"))

            def acc_tile():
                return ps_pool.tile([128, 512], F32, name="acc", tag="acc", bufs=4)

            def sp_tile():
                return ps_pool.tile([128, 1024], F32, name="spt", tag="sp", bufs=2)

            # ---------------- projections ----------------
            with ExitStack() as proj:
                xt_pool = proj.enter_context(tc.tile_pool(name="xtp", bufs=1))

                def psum_quad(flip):
                    # alternate between the acc and sp PSUM tags so one
                    # projection accumulates while DVE rope / ScalarE copy
                    # still drains the previous one's PSUM
                    if flip:
                        s0, s1 = sp_tile(), sp_tile()
                        return [s0[:, :512], s0[:, 512:], s1[:, :512], s1[:, 512:]]
                    return [acc_tile()[:] for _ in range(TB)]

                # ---- V: both kv heads projected chunk-paired so the PE
                # consumes each fresh xt chunk twice and never outruns the
                # initial xt DMA stream ----
                with ExitStack() as vstk:
                    vw_pool = vstk.enter_context(tc.tile_pool(name="vwp", bufs=8))
                    raw_pool = vstk.enter_context(tc.tile_pool(name="rawp", bufs=2))
                    ws_v = [None] * (NKV_L * 4)

                    def load_vw(f, qtr):
                        wt = vw_pool.tile([128, 8, 128], BF16, name="vwt", tag="vwt")
                        nc.sync.dma_start(
                            out=wt[:], in_=wv_d[f, :, qtr * 8:(qtr + 1) * 8, :])
                        ws_v[f * 4 + qtr] = wt

                    # first quarter of each V head, then the first xt chunks,
                    # then the rest — so the first matmul starts ~3us in
                    load_vw(0, 0)
                    load_vw(1, 0)
                    xt_first = []
                    for i in range(2):
                        t = xt_pool.tile([128, 1, S], BF16, name=f"xtf{i}", tag=f"xtf{i}")
                        nc.sync.dma_start(out=t[:], in_=xt_d[:, i:i + 1, :])
                        xt_first.append(t)
                    for qtr in range(1, 4):
                        load_vw(0, qtr)
                        load_vw(1, qtr)
                    xts = []
                    for i in range(1, 16):
                        t = xt_pool.tile([128, 2, S], BF16, name=f"xtt{i}", tag=f"xtt{i}")
                        nc.sync.dma_start(out=t[:], in_=xt_d[:, i * 2:(i + 1) * 2, :])
                        xts.append(t)

                    def xt_ap(hc, lo, hi):
                        if hc < 2:
                            return xt_first[hc][:, 0, lo:hi]
                        return xts[hc // 2 - 1][:, hc % 2, lo:hi]

                    pss_v = [psum_quad(False), psum_quad(True)]
                    for hc in range(HC):
                        for f in range(NKV_L):
                            lhsT = ws_v[f * 4 + hc // 8][:, hc % 8, :]
                            for tb in range(TB):
                                nc.tensor.matmul(
                                    pss_v[f][tb], lhsT,
                                    xt_ap(hc, tb * 512, (tb + 1) * 512),
                                    start=(hc == 0), stop=(hc == HC - 1),
                                )
                    for f in range(NKV_L):
                        raw = raw_pool.tile([128, S], BF16, name="raw", tag="raw")
                        for tb in range(TB):
                            # split the PSUM drain across scalar and vector so
                            # the V transposes can start sooner
                            if f == 0:
                                nc.scalar.copy(raw[:, tb * 512:(tb + 1) * 512], pss_v[f][tb])
                            else:
                                nc.vector.tensor_copy(raw[:, tb * 512:(tb + 1) * 512], pss_v[f][tb])
                        nc.sync.dma_start_transpose(
                            out=v_all[:, :, f * 128:(f + 1) * 128], in_=raw[:],
                        )

                # ---- K and Q, with RoPE applied straight from PSUM ----
                with ExitStack() as kq:
                    # K weights + rope tables on the idle gpsimd (SWDGE)
                    # queue; Q weights on the scalar HWDGE queue (naturally
                    # paced behind the V psum drain) — keeping them all off
                    # the sync queue avoids head-of-line blocking behind the
                    # xt stream and V transposes
                    cs_pool = kq.enter_context(tc.tile_pool(name="csp", bufs=1))
                    cos_sb = cs_pool.tile([128, S], BF16, name="cos_sb", tag="cos_sb")
                    sin_sb = cs_pool.tile([128, S], BF16, name="sin_sb", tag="sin_sb")
                    nc.gpsimd.dma_start(out=cos_sb[:], in_=cos_d[:])
                    nc.gpsimd.dma_start(out=sin_sb[:], in_=sin2_d[:])
                    wpool = kq.enter_context(tc.tile_pool(name="wpool", bufs=4))
                    tmp_pool = kq.enter_context(tc.tile_pool(name="tmpp", bufs=2))

                    def project_T(w_d, f, flip, w_eng):
                        ws = []
                        for qtr in range(4):
                            wt = wpool.tile([128, 8, 128], BF16, name="wt", tag="wt")
                            w_eng.dma_start(
                                out=wt[:], in_=w_d[f, :, qtr * 8:(qtr + 1) * 8, :])
                            ws.append(wt)
                        pss = psum_quad(flip)
                        for hc in range(HC):
                            lhsT = ws[hc // 8][:, hc % 8, :]
                            for tb in range(TB):
                                nc.tensor.matmul(
                                    pss[tb], lhsT, xt_ap(hc, tb * 512, (tb + 1) * 512),
                                    start=(hc == 0), stop=(hc == HC - 1),
                                )
                        return pss

                    def rope_into(pss, dst, idx):
                        # dst = psum*cos + rot_half(psum)*sin; rot_half via
                        # partition-shifted reads of the PSUM tile against the
                        # sign-folded sin table (legal: one operand in PSUM)
                        for tb in range(TB):
                            sl = slice(tb * 512, (tb + 1) * 512)
                            ps = pss[tb]
                            t1 = tmp_pool.tile([128, 512], F32, name="t1", tag="t1")
                            t2 = tmp_pool.tile([128, 512], F32, name="t2", tag="t2")
                            nc.vector.tensor_mul(t1[:], ps, cos_sb[:, sl])
                            nc.vector.tensor_mul(t2[0:64, :], ps[64:128, :], sin_sb[0:64, sl])
                            nc.vector.tensor_mul(t2[64:128, :], ps[0:64, :], sin_sb[64:128, sl])
                            nc.vector.tensor_add(dst[:, idx, sl], t1[:], t2[:])

                    flip = False
                    for f in range(NKV_L):
                        rope_into(project_T(wk_d, f, flip, nc.gpsimd), k_all, f)
                        flip = not flip
                    for f in range(NH_L):
                        rope_into(project_T(wq_d, f, flip, nc.scalar), q_all, f)
                        flip = not flip

            # ---------------- attention + output projection ----------------
            with ExitStack() as att:
                wo_pool = att.enter_context(tc.tile_pool(name="wop", bufs=1))
                wo_sb = wo_pool.tile([128, NH_L, H], BF16, name="wo_sb", tag="wo_sb")
                for h in range(NH_L):
                    # SWDGE queues: keep the HWDGE queues free for the
                    # latency-critical P^T transposes
                    nc.gpsimd.dma_start(out=wo_sb[:, h, :], in_=wo_d[h])

                ct_pool = att.enter_context(tc.tile_pool(name="ctp", bufs=2))
                rs_pool = att.enter_context(tc.tile_pool(name="rsp", bufs=8))
                osb_pool = att.enter_context(tc.tile_pool(name="osbp", bufs=2))

                p_pool = att.enter_context(tc.tile_pool(name="pp", bufs=2))
                ptt_pool = att.enter_context(tc.tile_pool(name="pttp", bufs=2))

                cts_by_qb = {}

                def softmax_part(qb, h):
                    """QK + mask + exp + row-normalize + one merged P^T DMA."""
                    kv = h // (NH_L // NKV_L)
                    nkb = qb + 1
                    W = nkb * 512
                    p_sb = p_pool.tile([128, 4, W], BF16, name="p_sb", tag="p")
                    ptt = ptt_pool.tile([128, 4, nkb * 4, 128], BF16,
                                        name="ptt", tag="ptt")
                    for qcl in range(4):
                        qg = 4 * qb + qcl
                        Wq = (qg + 1) * 128
                        rs = rs_pool.tile([128, 2], F32, name="rs", tag="rs")
                        for kb2 in range(0, nkb, 2):
                            sp = sp_tile()
                            w0 = 0
                            for j in (0, 1):
                                kb = kb2 + j
                                if kb >= nkb:
                                    break
                                wk_ = 512 if kb < qb else Wq - kb * 512
                                nc.tensor.matmul(
                                    sp[:, j * 512:j * 512 + wk_],
                                    q_all[:, h, qg * 128:(qg + 1) * 128],
                                    k_all[:, kv, kb * 512:kb * 512 + wk_],
                                    start=True, stop=True,
                                )
                                if kb == qb:
                                    # additive causal mask for the diagonal
                                    # 128-block, applied on DVE into PSUM
                                    dsl = slice(j * 512 + wk_ - 128, j * 512 + wk_)
                                    nc.vector.tensor_add(sp[:, dsl], sp[:, dsl], tria_sb[:])
                                w0 += wk_
                            nc.scalar.activation(
                                p_sb[:, qcl, kb2 * 512:kb2 * 512 + w0], sp[:, :w0], EXP,
                                bias=bias0[:], scale=SCALE,
                                accum_out=rs[:, kb2 // 2:kb2 // 2 + 1],
                            )
                        nacc = (nkb + 1) // 2
                        rq = rs_pool.tile([128, 1], F32, name="rq", tag="rq")
                        if nacc > 1:
                            dsum = rs_pool.tile([128, 1], F32, name="dsum", tag="dsum")
                            nc.vector.tensor_reduce(dsum[:], rs[:, :nacc], axis=AX, op=ADD)
                            nc.vector.reciprocal(rq[:], dsum[:])
                        else:
                            nc.vector.reciprocal(rq[:], rs[:, :1])
                        nc.vector.tensor_scalar_mul(
                            p_sb[:, qcl, :Wq], p_sb[:, qcl, :Wq], rq[:])
                    # one merged transpose for all 4 q-chunks; chunks beyond a
                    # chunk's causal width hold stale garbage that the AV
                    # narrowing never reads
                    nc.sync.dma_start_transpose(out=ptt[:], in_=p_sb[:])
                    return ptt

                def av_part(qb, h, ptt):
                    kv = h // (NH_L // NKV_L)
                    nkc = 4 * (qb + 1)
                    av = acc_tile()
                    for kc in range(nkc):
                        d = max(0, kc - 4 * qb)
                        off = d * 128
                        nc.tensor.matmul(
                            av[:, off:512],
                            v_all[:, kc, kv * 128:(kv + 1) * 128],
                            ptt[:, d:4, kc, :],
                            start=(kc == 0), stop=(kc == nkc - 1),
                        )
                    nc.vector.tensor_copy(cts_by_qb[qb][:, h, :], av[:])

                osb_cur = [None]

                def oproj_group(qb, qcl, oc):
                    cts = cts_by_qb[qb]
                    qc = qb * 4 + qcl
                    op = acc_tile()
                    for h in range(NH_L):
                        nc.tensor.matmul(
                            op[:],
                            cts[:, h, qcl * 128:(qcl + 1) * 128],
                            wo_sb[:, h, oc * 512:(oc + 1) * 512],
                            start=(h == 0), stop=(h == NH_L - 1),
                        )
                    # batch 4 oc groups into one bf16 tile -> one out-DMA
                    if oc % 4 == 0:
                        osb_cur[0] = osb_pool.tile(
                            [128, 4, 512], BF16, name="osb", tag="osb")
                    osb = osb_cur[0]
                    nc.vector.tensor_copy(osb[:, oc % 4, :], op[:])
                    if oc % 4 == 3:
                        nc.gpsimd.dma_start(
                            out=out_d[qc * 128:(qc + 1) * 128,
                                      (oc - 3) * 512:(oc + 1) * 512],
                            in_=osb[:],
                        )

                # Software pipeline: softmax(i) runs 2 heads ahead of av(i);
                # o-projection groups of the previous (already AV-complete)
                # q-block drip between steps as PE filler so the exp ->
                # normalize -> P^T transpose chain never drains the PE.
                pairs = [(qb, h) for qb in (0, 1, 2, 3) for h in range(NH_L)]
                LOOK = 2
                DRIP = 4
                RESERVE = 4   # groups held back to pad the tail avs
                ptts = {}
                oproj_queue = []

                def emit_av(i):
                    qb, h = pairs[i]
                    av_part(qb, h, ptts.pop(i))
                    if h == NH_L - 1:
                        for qcl in range(4):
                            for oc in range(OC):
                                oproj_queue.append((qb, qcl, oc))

                def drip(n):
                    for _ in range(min(n, len(oproj_queue))):
                        pq, qcl, oc = oproj_queue.pop(0)
                        oproj_group(pq, qcl, oc)

                # per step: av(i-LOOK) + oproj filler FIRST on the PE queue,
                # then this step's QK — so the P^T buffer ring (which ties
                # tp(i) to av(i-LOOK)) unblocks early in the step instead of
                # stalling the transpose queue behind the whole step
                last_block = len(pairs) - NH_L
                for i, (qb, h) in enumerate(pairs):
                    if h == 0:
                        cts_by_qb[qb] = ct_pool.tile(
                            [128, NH_L, 512], BF16, name="cts", tag="ct")
                    if i >= LOOK:
                        emit_av(i - LOOK)
                    ptts[i] = softmax_part(qb, h)
                    if i >= LOOK:
                        budget = DRIP
                        if i >= last_block:
                            budget = min(budget, len(oproj_queue) - RESERVE)
                        drip(max(0, budget))
                for i in range(len(pairs) - LOOK, len(pairs)):
                    emit_av(i)
                    drip(RESERVE // LOOK)
                while oproj_queue:
                    pq, qcl, oc = oproj_queue.pop(0)
                    oproj_group(pq, qcl, oc)
    nc.compile()
    return nc


def _prep_inputs(hidden_states, position_ids, Wq, Wk, Wv, Wo):
    bf = ml_dtypes.bfloat16
    hidden_states = np.asarray(hidden_states, dtype=np.float32)
    position_ids = np.asarray(position_ids)
    Wq = np.asarray(Wq, dtype=np.float32)
    Wk = np.asarray(Wk, dtype=np.float32)
    Wv = np.asarray(Wv, dtype=np.float32)
    Wo = np.asarray(Wo, dtype=np.float32)

    inv_freq = (1.0 / (BASE ** (np.arange(0, HD, 2, dtype=np.float32) / HD))).astype(np.float32)
    ii = np.arange(128)
    tria = np.where(ii[None, :] > ii[:, None], np.float32(NEG), np.float32(0.0))

    per_batch = []
    for b in range(B):
        xt = np.ascontiguousarray(
            hidden_states[b].T.reshape(HC, 128, S).transpose(1, 0, 2)
        ).astype(bf)
        pos = position_ids[b].astype(np.float32)
        freqs = pos[:, None] * inv_freq[None, :]           # [S, 64]
        emb = np.concatenate([freqs, freqs], axis=1)       # [S, 128]
        cos = np.ascontiguousarray(np.cos(emb).T).astype(bf)
        sin = np.sin(emb).T                                # [128, S] f32
        sin2 = sin.copy()
        sin2[:64] = -sin[:64]                              # sign-folded
        sin2 = np.ascontiguousarray(sin2).astype(bf)
        per_batch.append((xt, cos, sin2))

    in_maps = []
    for core in range(N_CORES):
        b, g = core // GROUPS, core % GROUPS
        xt, cos, sin2 = per_batch[b]
        wq = np.ascontiguousarray(
            Wq[:, g * NH_L * HD:(g + 1) * NH_L * HD]
            .reshape(HC, 128, NH_L, 128).transpose(2, 1, 0, 3)
        ).astype(bf)
        wk = np.ascontiguousarray(
            Wk[:, g * NKV_L * HD:(g + 1) * NKV_L * HD]
            .reshape(HC, 128, NKV_L, 128).transpose(2, 1, 0, 3)
        ).astype(bf)
        wv = np.ascontiguousarray(
            Wv[:, g * NKV_L * HD:(g + 1) * NKV_L * HD]
            .reshape(HC, 128, NKV_L, 128).transpose(2, 1, 0, 3)
        ).astype(bf)
        wo = np.ascontiguousarray(
            Wo[g * NH_L * HD:(g + 1) * NH_L * HD, :].reshape(NH_L, 128, H)
        ).astype(bf)
        in_maps.append({
            "xt": xt, "wq": wq, "wk": wk, "wv": wv, "wo": wo,
            "cos": cos, "sin2": sin2, "tria": tria,
        })
    return in_maps


def kernel(hidden_states, position_ids, Wq, Wk, Wv, Wo):
    global _PROG, LAST_EXEC_NS, LAST_RESULTS
    if _PROG is None:
        _PROG = _build()
    nc = _PROG
    in_maps = _prep_inputs(hidden_states, position_ids, Wq, Wk, Wv, Wo)
    trace = os.environ.get("BASS_KERNEL_TRACE", "0") == "1"
    res = run_bass_kernel_spmd(nc, in_maps, core_ids=list(range(N_CORES)), trace=trace)
    LAST_EXEC_NS = res.exec_time_ns
    LAST_RESULTS = res
    out = np.zeros((B, S, H), dtype=np.float32)
    for core in range(N_CORES):
        out[core // GROUPS] += res.results[core]["out_p"].astype(np.float32)
    return out


# revision 17
# speedup vs baseline: 1.1201x; 1.0263x over previous
"""GQA causal attention (RoPE) kernel for 8 TRN2 NeuronCores.

Sharding: core = b*4 + g  (b = batch 0..1, g = head-group 0..3).
Each core handles one batch element, 8 query heads (g*8..g*8+7) and the
2 KV heads (g*2, g*2+1) that serve them, plus the matching row-block of
Wo; per-core outputs are partial sums over the hidden dim that the host
reduces across the 4 groups of each batch.

On-core dataflow (all matmuls bf16 with f32 PSUM accumulation):
  QT = Wq_g.T @ X.T   [1024, 2048]   (feature-on-partition layout)
  KT = Wk_g.T @ X.T   [256, 2048]    + RoPE applied on DVE straight from
  VT = Wv_g.T @ X.T   PSUM (rotate_half = partition-shifted mul against a
                      sign-folded sin table); VT PE-free DMA-transposed to
                      V [2048, 256].
  per head, per q-128 chunk: S[q,k] = QT_chunk.T x KT (causal-narrowed),
  additive -1e9 mask on the diagonal 128-block via a DVE add into PSUM,
  P = exp(S/sqrt(d)) on ScalarE with fused row-sum (accum_out) -> per-row
  reciprocal -> P normalized in place; all 4 q-chunks of a head go out in
  ONE merged DMA transpose to P^T; ctx^T = V.T @ P^T; out_partial =
  ctx @ Wo_g (row block) accumulated over heads in PSUM.

Software pipeline: softmax(i) runs 2 heads ahead of AV(i); o-projection
groups of the previous q-block drip between AVs so the PE never drains
while a softmax chain (exp -> normalize -> P^T transpose) is in flight.
"""

import os

import numpy as np
import ml_dtypes

import concourse.bass as bass
import concourse.mybir as mybir
import concourse.tile as tile
from concourse import bacc
from concourse.bass_utils import run_bass_kernel_spmd
from contextlib import ExitStack

B, S, H = 2, 2048, 4096
NH, NKV, HD = 32, 8, 128
BASE = 10000.0
N_CORES = 8
GROUPS = 4
NH_L = NH // GROUPS        # 8 local q heads
NKV_L = NKV // GROUPS      # 2 local kv heads
HC = H // 128              # 32 hidden chunks
TC = S // 128              # 16 token chunks
TB = S // 512              # 4 token 512-blocks
OC = H // 512              # 8 output-feature 512-blocks
SCALE = 1.0 / float(np.sqrt(HD))
NEG = -1e9

BF16 = mybir.dt.bfloat16
F32 = mybir.dt.float32
EXP = mybir.ActivationFunctionType.Exp
AX = mybir.AxisListType.X
ADD = mybir.AluOpType.add

_PROG = None
LAST_EXEC_NS = None
LAST_RESULTS = None


def _build():
    nc = bacc.Bacc(None, target_bir_lowering=False, debug=False)
    with tile.TileContext(nc) as tc:
        xt_d = nc.dram_tensor("xt", [128, HC, S], BF16, kind="ExternalInput")
        wq_d = nc.dram_tensor("wq", [NH_L, 128, HC, 128], BF16, kind="ExternalInput")
        wk_d = nc.dram_tensor("wk", [NKV_L, 128, HC, 128], BF16, kind="ExternalInput")
        wv_d = nc.dram_tensor("wv", [NKV_L, 128, HC, 128], BF16, kind="ExternalInput")
        wo_d = nc.dram_tensor("wo", [NH_L, 128, H], BF16, kind="ExternalInput")
        cos_d = nc.dram_tensor("cos", [128, S], BF16, kind="ExternalInput")
        # sin2: rows 0-63 hold -sin[0:64], rows 64-127 hold sin[64:128], so
        # rotate_half(x)*sin == shiftedmul(x, sin2) with partition offsets
        sin2_d = nc.dram_tensor("sin2", [128, S], BF16, kind="ExternalInput")
        tria_d = nc.dram_tensor("tria", [128, 128], F32, kind="ExternalInput")
        out_d = nc.dram_tensor("out_p", [S, H], BF16, kind="ExternalOutput")

        with ExitStack() as stk:
            persist = stk.enter_context(tc.tile_pool(name="persist", bufs=1))
            q_all = persist.tile([128, NH_L, S], BF16, name="q_all", tag="q_all")
            k_all = persist.tile([128, NKV_L, S], BF16, name="k_all", tag="k_all")
            v_all = persist.tile([128, TC, NKV_L * 128], BF16, name="v_all", tag="v_all")
            tria_sb = persist.tile([128, 128], F32, name="tria_sb", tag="tria_sb")
            bias0 = persist.tile([128, 1], F32, name="bias0", tag="bias0")

            nc.sync.dma_start(out=tria_sb[:], in_=tria_d[:])
            nc.any.memset(bias0[:], 0.0)

            # one PSUM pool for the whole program:
            #   tag "acc" (4 banks): projection accumulators, attention AV,
            #                        o-proj accumulators
            #   tag "sp"  (4 banks): QK scores (two 512 k-blocks share one
            #                        tile so exp runs as one wide instruction)
            ps_pool = stk.enter_context(tc.tile_pool(name="ps", bufs=1, space="PSUM"))

            def acc_tile():
                return ps_pool.tile([128, 512], F32, name="acc", tag="acc", bufs=4)

            def sp_tile():
                return ps_pool.tile([128, 1024], F32, name="spt", tag="sp", bufs=2)

            # ---------------- projections ----------------
            with ExitStack() as proj:
                xt_pool = proj.enter_context(tc.tile_pool(name="xtp", bufs=1))
                # K/Q weight quarters stream through one just-in-time ring on
                # the sync queue; emitted before the V psum-drain so the V
                # transposes can never head-of-line-block them
                wpool = proj.enter_context(tc.tile_pool(name="wpool", bufs=4))

                def psum_quad(flip):
                    # alternate between the acc and sp PSUM tags so one
                    # projection accumulates while DVE rope / ScalarE copy
                    # still drains the previous one's PSUM
                    if flip:
                        s0, s1 = sp_tile(), sp_tile()
                        return [s0[:, :512], s0[:, 512:], s1[:, :512], s1[:, 512:]]
                    return [acc_tile()[:] for _ in range(TB)]

                def load_quarter(w_d, f, qtr):
                    wt = wpool.tile([128, 8, 128], BF16, name="wt", tag="wt")
                    nc.sync.dma_start(
                        out=wt[:], in_=w_d[f, :, qtr * 8:(qtr + 1) * 8, :])
                    return wt

                # ---- V: both kv heads projected chunk-paired so the PE
                # consumes each fresh xt chunk twice and never outruns the
                # initial xt DMA stream; V weight quarters interleave with
                # the first xt tiles ----
                with ExitStack() as vstk:
                    vw_pool = vstk.enter_context(tc.tile_pool(name="vwp", bufs=8))
                    ws_v = [None] * (NKV_L * 4)

                    def load_vw(f, qtr):
                        wt = vw_pool.tile([128, 8, 128], BF16, name="vwt", tag="vwt")
                        nc.sync.dma_start(
                            out=wt[:], in_=wv_d[f, :, qtr * 8:(qtr + 1) * 8, :])
                        ws_v[f * 4 + qtr] = wt

                    load_vw(0, 0)
                    load_vw(1, 0)
                    wk_q0 = load_quarter(wk_d, 0, 0)
                    xt_first = []
                    for i in range(2):
                        t = xt_pool.tile([128, 1, S], BF16, name=f"xtf{i}", tag=f"xtf{i}")
                        nc.sync.dma_start(out=t[:], in_=xt_d[:, i:i + 1, :])
                        xt_first.append(t)
                    xts = []
                    for i in range(1, 16):
                        t = xt_pool.tile([128, 2, S], BF16, name=f"xtt{i}", tag=f"xtt{i}")
                        nc.sync.dma_start(out=t[:], in_=xt_d[:, i * 2:(i + 1) * 2, :])
                        xts.append(t)
                        if i <= 3:  # interleave remaining V quarters early
                            load_vw(0, i)
                            load_vw(1, i)

                    def xt_ap(hc, lo, hi):
                        if hc < 2:
                            return xt_first[hc][:, 0, lo:hi]
                        return xts[hc // 2 - 1][:, hc % 2, lo:hi]

                    pss_v = [psum_quad(False), psum_quad(True)]
                    for hc in range(HC):
                        for f in range(NKV_L):
                            lhsT = ws_v[f * 4 + hc // 8][:, hc % 8, :]
                            for tb in range(TB):
                                nc.tensor.matmul(
                                    pss_v[f][tb], lhsT,
                                    xt_ap(hc, tb * 512, (tb + 1) * 512),
                                    start=(hc == 0), stop=(hc == HC - 1),
                                )

                # ---- K and Q, with RoPE applied straight from PSUM ----
                with ExitStack() as kq:
                    cs_pool = kq.enter_context(tc.tile_pool(name="csp", bufs=1))
                    cos_sb = cs_pool.tile([128, S], BF16, name="cos_sb", tag="cos_sb")
                    sin_sb = cs_pool.tile([128, S], BF16, name="sin_sb", tag="sin_sb")
                    raw_pool = kq.enter_context(tc.tile_pool(name="rawp", bufs=1))
                    tmp_pool = kq.enter_context(tc.tile_pool(name="tmpp", bufs=2))

                    # emit every remaining weight DMA now: the 4-deep ring
                    # paces each transfer behind the matmuls of the quarter
                    # it replaces, i.e. just-in-time prefetch
                    wts = [wk_q0]
                    for f in range(1, 4):
                        wts.append(load_quarter(wk_d, 0, f))
                    for f in range(4):
                        wts.append(load_quarter(wk_d, 1, f))
                    nc.sync.dma_start(out=cos_sb[:], in_=cos_d[:])
                    nc.sync.dma_start(out=sin_sb[:], in_=sin2_d[:])
                    for f in range(NH_L):
                        for qtr in range(4):
                            wts.append(load_quarter(wq_d, f, qtr))

                    # V psum drain (split across scalar and vector) + PE-free
                    # DMA transposes; v_all is not needed until attention
                    for f in range(NKV_L):
                        raw = raw_pool.tile([128, S], BF16, name="raw", tag="raw")
                        for tb in range(TB):
                            if f == 0:
                                nc.scalar.copy(raw[:, tb * 512:(tb + 1) * 512], pss_v[f][tb])
                            else:
                                nc.vector.tensor_copy(raw[:, tb * 512:(tb + 1) * 512], pss_v[f][tb])
                        nc.sync.dma_start_transpose(
                            out=v_all[:, :, f * 128:(f + 1) * 128], in_=raw[:],
                        )

                    def project_T(ws, flip):
                        pss = psum_quad(flip)
                        for hc in range(HC):
                            lhsT = ws[hc // 8][:, hc % 8, :]
                            for tb in range(TB):
                                nc.tensor.matmul(
                                    pss[tb], lhsT, xt_ap(hc, tb * 512, (tb + 1) * 512),
                                    start=(hc == 0), stop=(hc == HC - 1),
                                )
                        return pss

                    def rope_into(pss, dst, idx):
                        # dst = psum*cos + rot_half(psum)*sin; rot_half via
                        # partition-shifted reads of the PSUM tile against the
                        # sign-folded sin table (legal: one operand in PSUM)
                        for tb in range(TB):
                            sl = slice(tb * 512, (tb + 1) * 512)
                            ps = pss[tb]
                            t1 = tmp_pool.tile([128, 512], F32, name="t1", tag="t1")
                            t2 = tmp_pool.tile([128, 512], F32, name="t2", tag="t2")
                            nc.vector.tensor_mul(t1[:], ps, cos_sb[:, sl])
                            nc.vector.tensor_mul(t2[0:64, :], ps[64:128, :], sin_sb[0:64, sl])
                            nc.vector.tensor_mul(t2[64:128, :], ps[0:64, :], sin_sb[64:128, sl])
                            nc.vector.tensor_add(dst[:, idx, sl], t1[:], t2[:])

                    flip = False
                    for f in range(NKV_L):
                        rope_into(project_T(wts[4 * f:4 * f + 4], flip), k_all, f)
                        flip = not flip
                    for f in range(NH_L):
                        rope_into(project_T(wts[8 + 4 * f:12 + 4 * f], flip), q_all, f)
                        flip = not flip

            # ---------------- attention + output projection ----------------
            with ExitStack() as att:
                wo_pool = att.enter_context(tc.tile_pool(name="wop", bufs=1))
                wo_sb = wo_pool.tile([128, NH_L, H], BF16, name="wo_sb", tag="wo_sb")
                for h in range(NH_L):
                    # SWDGE queues: keep the HWDGE queues free for the
                    # latency-critical P^T transposes
                    nc.gpsimd.dma_start(out=wo_sb[:, h, :], in_=wo_d[h])

                ct_pool = att.enter_context(tc.tile_pool(name="ctp", bufs=2))
                rs_pool = att.enter_context(tc.tile_pool(name="rsp", bufs=8))
                osb_pool = att.enter_context(tc.tile_pool(name="osbp", bufs=2))

                p_pool = att.enter_context(tc.tile_pool(name="pp", bufs=2))
                ptt_pool = att.enter_context(tc.tile_pool(name="pttp", bufs=2))

                cts_by_qb = {}

                def softmax_part(qb, h):
                    """QK + mask + exp + row-normalize + one merged P^T DMA."""
                    kv = h // (NH_L // NKV_L)
                    nkb = qb + 1
                    W = nkb * 512
                    p_sb = p_pool.tile([128, 4, W], BF16, name="p_sb", tag="p")
                    ptt = ptt_pool.tile([128, 4, nkb * 4, 128], BF16,
                                        name="ptt", tag="ptt")
                    for qcl in range(4):
                        qg = 4 * qb + qcl
                        Wq = (qg + 1) * 128
                        rs = rs_pool.tile([128, 2], F32, name="rs", tag="rs")
                        for kb2 in range(0, nkb, 2):
                            sp = sp_tile()
                            w0 = 0
                            for j in (0, 1):
                                kb = kb2 + j
                                if kb >= nkb:
                                    break
                                wk_ = 512 if kb < qb else Wq - kb * 512
                                nc.tensor.matmul(
                                    sp[:, j * 512:j * 512 + wk_],
                                    q_all[:, h, qg * 128:(qg + 1) * 128],
                                    k_all[:, kv, kb * 512:kb * 512 + wk_],
                                    start=True, stop=True,
                                )
                                if kb == qb:
                                    # additive causal mask for the diagonal
                                    # 128-block, applied on DVE into PSUM
                                    dsl = slice(j * 512 + wk_ - 128, j * 512 + wk_)
                                    nc.vector.tensor_add(sp[:, dsl], sp[:, dsl], tria_sb[:])
                                w0 += wk_
                            nc.scalar.activation(
                                p_sb[:, qcl, kb2 * 512:kb2 * 512 + w0], sp[:, :w0], EXP,
                                bias=bias0[:], scale=SCALE,
                                accum_out=rs[:, kb2 // 2:kb2 // 2 + 1],
                            )
                        nacc = (nkb + 1) // 2
                        rq = rs_pool.tile([128, 1], F32, name="rq", tag="rq")
                        if nacc > 1:
                            dsum = rs_pool.tile([128, 1], F32, name="dsum", tag="dsum")
                            nc.vector.tensor_reduce(dsum[:], rs[:, :nacc], axis=AX, op=ADD)
                            nc.vector.reciprocal(rq[:], dsum[:])
                        else:
                            nc.vector.reciprocal(rq[:], rs[:, :1])
                        nc.vector.tensor_scalar_mul(
                            p_sb[:, qcl, :Wq], p_sb[:, qcl, :Wq], rq[:])
                    # one merged transpose for all 4 q-chunks; chunks beyond a
                    # chunk's causal width hold stale garbage that the AV
                    # narrowing never reads
                    nc.sync.dma_start_transpose(out=ptt[:], in_=p_sb[:])
                    return ptt

                def av_part(qb, h, ptt):
                    kv = h // (NH_L // NKV_L)
                    nkc = 4 * (qb + 1)
                    av = acc_tile()
                    for kc in range(nkc):
                        d = max(0, kc - 4 * qb)
                        off = d * 128
                        nc.tensor.matmul(
                            av[:, off:512],
                            v_all[:, kc, kv * 128:(kv + 1) * 128],
                            ptt[:, d:4, kc, :],
                            start=(kc == 0), stop=(kc == nkc - 1),
                        )
                    nc.vector.tensor_copy(cts_by_qb[qb][:, h, :], av[:])

                osb_cur = [None]

                def oproj_group(qb, qcl, oc):
                    cts = cts_by_qb[qb]
                    qc = qb * 4 + qcl
                    op = acc_tile()
                    for h in range(NH_L):
                        nc.tensor.matmul(
                            op[:],
                            cts[:, h, qcl * 128:(qcl + 1) * 128],
                            wo_sb[:, h, oc * 512:(oc + 1) * 512],
                            start=(h == 0), stop=(h == NH_L - 1),
                        )
                    # batch 4 oc groups into one bf16 tile -> one out-DMA
                    if oc % 4 == 0:
                        osb_cur[0] = osb_pool.tile(
                            [128, 4, 512], BF16, name="osb", tag="osb")
                    osb = osb_cur[0]
                    nc.vector.tensor_copy(osb[:, oc % 4, :], op[:])
                    if oc % 4 == 3:
                        nc.gpsimd.dma_start(
                            out=out_d[qc * 128:(qc + 1) * 128,
                                      (oc - 3) * 512:(oc + 1) * 512],
                            in_=osb[:],
                        )

                # Software pipeline: softmax(i) runs 2 heads ahead of av(i);
                # o-projection groups of the previous (already AV-complete)
                # q-block drip between steps as PE filler so the exp ->
                # normalize -> P^T transpose chain never drains the PE.
                pairs = [(qb, h) for qb in (0, 1, 2, 3) for h in range(NH_L)]
                LOOK = 2
                DRIP = 4
                RESERVE = 4   # groups held back to pad the tail avs
                ptts = {}
                oproj_queue = []

                def emit_av(i):
                    qb, h = pairs[i]
                    av_part(qb, h, ptts.pop(i))
                    if h == NH_L - 1:
                        for qcl in range(4):
                            for oc in range(OC):
                                oproj_queue.append((qb, qcl, oc))

                def drip(n):
                    for _ in range(min(n, len(oproj_queue))):
                        pq, qcl, oc = oproj_queue.pop(0)
                        oproj_group(pq, qcl, oc)

                # per step: av(i-LOOK) + oproj filler FIRST on the PE queue,
                # then this step's QK — so the P^T buffer ring (which ties
                # tp(i) to av(i-LOOK)) unblocks early in the step instead of
                # stalling the transpose queue behind the whole step
                last_block = len(pairs) - NH_L
                for i, (qb, h) in enumerate(pairs):
                    if h == 0:
                        cts_by_qb[qb] = ct_pool.tile(
                            [128, NH_L, 512], BF16, name="cts", tag="ct")
                    if i >= LOOK:
                        emit_av(i - LOOK)
                    ptts[i] = softmax_part(qb, h)
                    if i >= LOOK:
                        budget = DRIP
                        if i >= last_block:
                            budget = min(budget, len(oproj_queue) - RESERVE)
                        drip(max(0, budget))
                for i in range(len(pairs) - LOOK, len(pairs)):
                    emit_av(i)
                    drip(RESERVE // LOOK)
                while oproj_queue:
                    pq, qcl, oc = oproj_queue.pop(0)
                    oproj_group(pq, qcl, oc)
    nc.compile()
    return nc


def _prep_inputs(hidden_states, position_ids, Wq, Wk, Wv, Wo):
    bf = ml_dtypes.bfloat16
    hidden_states = np.asarray(hidden_states, dtype=np.float32)
    position_ids = np.asarray(position_ids)
    Wq = np.asarray(Wq, dtype=np.float32)
    Wk = np.asarray(Wk, dtype=np.float32)
    Wv = np.asarray(Wv, dtype=np.float32)
    Wo = np.asarray(Wo, dtype=np.float32)

    inv_freq = (1.0 / (BASE ** (np.arange(0, HD, 2, dtype=np.float32) / HD))).astype(np.float32)
    ii = np.arange(128)
    tria = np.where(ii[None, :] > ii[:, None], np.float32(NEG), np.float32(0.0))

    per_batch = []
    for b in range(B):
        xt = np.ascontiguousarray(
            hidden_states[b].T.reshape(HC, 128, S).transpose(1, 0, 2)
        ).astype(bf)
        pos = position_ids[b].astype(np.float32)
        freqs = pos[:, None] * inv_freq[None, :]           # [S, 64]
        emb = np.concatenate([freqs, freqs], axis=1)       # [S, 128]
        cos = np.ascontiguousarray(np.cos(emb).T).astype(bf)
        sin = np.sin(emb).T                                # [128, S] f32
        sin2 = sin.copy()
        sin2[:64] = -sin[:64]                              # sign-folded
        sin2 = np.ascontiguousarray(sin2).astype(bf)
        per_batch.append((xt, cos, sin2))

    in_maps = []
    for core in range(N_CORES):
        b, g = core // GROUPS, core % GROUPS
        xt, cos, sin2 = per_batch[b]
        wq = np.ascontiguousarray(
            Wq[:, g * NH_L * HD:(g + 1) * NH_L * HD]
            .reshape(HC, 128, NH_L, 128).transpose(2, 1, 0, 3)
        ).astype(bf)
        wk = np.ascontiguousarray(
            Wk[:, g * NKV_L * HD:(g + 1) * NKV_L * HD]
            .reshape(HC, 128, NKV_L, 128).transpose(2, 1, 0, 3)
        ).astype(bf)
        wv = np.ascontiguousarray(
            Wv[:, g * NKV_L * HD:(g + 1) * NKV_L * HD]
            .reshape(HC, 128, NKV_L, 128).transpose(2, 1, 0, 3)
        ).astype(bf)
        wo = np.ascontiguousarray(
            Wo[g * NH_L * HD:(g + 1) * NH_L * HD, :].reshape(NH_L, 128, H)
        ).astype(bf)
        in_maps.append({
            "xt": xt, "wq": wq, "wk": wk, "wv": wv, "wo": wo,
            "cos": cos, "sin2": sin2, "tria": tria,
        })
    return in_maps


def kernel(hidden_states, position_ids, Wq, Wk, Wv, Wo):
    global _PROG, LAST_EXEC_NS, LAST_RESULTS
    if _PROG is None:
        _PROG = _build()
    nc = _PROG
    in_maps = _prep_inputs(hidden_states, position_ids, Wq, Wk, Wv, Wo)
    trace = os.environ.get("BASS_KERNEL_TRACE", "0") == "1"
    res = run_bass_kernel_spmd(nc, in_maps, core_ids=list(range(N_CORES)), trace=trace)
    LAST_EXEC_NS = res.exec_time_ns
    LAST_RESULTS = res
    out = np.zeros((B, S, H), dtype=np.float32)
    for core in range(N_CORES):
        out[core // GROUPS] += res.results[core]["out_p"].astype(np.float32)
    return out
